# revision 8
# baseline (speedup 1.0000x reference)
"""Trainium2 Bass kernel for nn_DebiasIntraDist (segment_reduce).

Full-input contract: kernel(**inputs) takes the complete (unsharded) inputs
and returns the full scalar loss. The N=65536 samples are sharded across the
8 NeuronCores by (demog, label-half): core 2d+h gets the rows with
demog == d and label-half h. Every core then owns a disjoint set of 256
(demog, label) groups, so no cross-core reduction is needed on device at
all: each core emits raw per-group accumulators and the host combines them
into the final loss (the "gather/unshard" step).

Within each shard the rows are ordered so that all rows whose group falls
in PSUM chunk 0 (local label < 128) come first, then chunk 1. Each 128-row
tile therefore feeds exactly ONE [128-group x D] PSUM accumulator.

The feats are rounded to bf16 ON THE HOST, halving HBM traffic (the DMA
stream is the kernel's roofline). All device math then computes the EXACT
loss of the bf16-rounded data: the one-hot matmuls contract exact bf16
values with fp32 accumulation. The bf16 rounding perturbs the final scalar
by ~1e-3 relative, far under the 2e-2 gate.

Math per core, per group g:
    cnt[g] (host bincount), sums[g, :], sumsq[g] = sum_{i in g} ||x_i||^2
    sum_{i in g} ||x_i - mu_g||^2 = sumsq[g] - ||sums[g]||^2 / cnt[g]

Each row is shipped as 514 bf16 values [x (512) | sq_hi | sq_lo], where
sq_hi + sq_lo is the exact-fp32 ||x||^2 of the bf16 row split into two
bf16 halves. Per tile the PE then runs TWO matmuls off the SAME one-hot
weights: [128x512] row sums into ps_sums and [128x2] group sumsq (hi, lo
accumulated separately in fp32 PSUM, so the split stays exact) into
ps_small. Group counts and the final ~10 scalar ops per group happen on
the host, which already knows the labels.
"""

import numpy as np

try:
    import concourse.bacc as bacc
except ImportError:  # fresh environment without PYTHONPATH set up
    import sys
    for p in ("/root/.axon_site/_ro/trn_rl_repo", "/opt/trn_rl_repo",
              "/root/.axon_site/_ro/pypackages"):
        if p not in sys.path:
            sys.path.append(p)
    import concourse.bacc as bacc
import concourse.mybir as mybir
import concourse.tile as tile
import concourse.bass_utils as bass_utils

N_CORES = 8
P = 128
D = 512          # feature dim
W = D + 2        # shipped row width: [x | sq_hi | sq_lo]
NL = 256         # labels per core after (demog, label-half) sharding
ND = 4           # demog values
NCH = NL // P    # one-hot chunks of 128 groups
CH = 6           # sample-tiles per feats DMA (~0.75 MiB in bf16)

_cache: dict[tuple, object] = {}


def _bf16(a: np.ndarray) -> np.ndarray:
    """Round-to-nearest-even fp32 -> bf16, returned as a uint16 view."""
    u = np.ascontiguousarray(a, dtype=np.float32).view(np.uint32)
    return ((u + 0x7FFF + ((u >> 16) & 1)) >> 16).astype(np.uint16)


def _build(key, debug: bool = False):
    """Compile the SPMD kernel for chunk tile counts (T0, T1)."""
    T0, T1 = key
    T = T0 + T1
    fp32 = mybir.dt.float32
    bf16 = mybir.dt.bfloat16
    Alu = mybir.AluOpType
    Act = mybir.ActivationFunctionType

    nc = bacc.Bacc("TRN2", target_bir_lowering=False, debug=False,
                   enable_asserts=False, num_devices=N_CORES)

    feats_t = nc.dram_tensor("feats_t", [P, T * W], bf16,
                             kind="ExternalInput").ap()
    # labels_t carries [labels | iota table] in one small early DMA
    labels_t = nc.dram_tensor("labels_t", [P, T + NL], fp32,
                              kind="ExternalInput").ap()
    nd_out = nc.dram_tensor("nd", [P, 6], fp32, kind="ExternalOutput").ap()

    chunk_start = (0, T0)
    chunk_stop = (T0 - 1, T - 1)

    # chunk schedule: small first chunk so compute starts ASAP; a short
    # final chunk keeps the compute tail after the last DMA short
    chunks = [(0, 1)]
    t = 1
    while t < T:
        L = min(CH, T - t)
        chunks.append((t, L))
        t += L
    if chunks[-1][1] > 2:
        t0, L = chunks.pop()
        chunks.append((t0, L - 2))
        chunks.append((t0 + L - 2, 2))

    with tile.TileContext(nc) as tc:
        with (
            tc.tile_pool(name="const", bufs=1) as constp,
            # one buffer per chunk: every dma_start issues right at the top
            # of the program (no buffer-reuse waits), so the DMA engines
            # stream back-to-back at full rate
            tc.tile_pool(name="fx", bufs=len(chunks)) as fxp,
            tc.tile_pool(name="oh16", bufs=6) as oh16p,
            tc.tile_pool(name="scr", bufs=2) as scrp,
            tc.tile_pool(name="post", bufs=1) as postp,
            tc.tile_pool(name="ps", bufs=1, space="PSUM") as psp,
            tc.tile_pool(name="dram", bufs=1, space="DRAM") as dramp,
        ):
            # per-group accumulators; each PSUM accumulation group owns a bank
            ps_sums = [psp.tile([P, D], fp32, tag=f"sums{c}", name=f"sums{c}")
                       for c in range(NCH)]
            ps_small = [psp.tile([P, 2], fp32, tag=f"small{c}",
                                 name=f"small{c}")
                        for c in range(NCH)]

            # the first NPRE chunks are DMA'd from the scalar queue right at
            # the top of the program (the sync engine sits in the NEFF
            # preamble slightly longer); later chunks alternate queues
            NPRE = min(3, len(chunks))
            labs = constp.tile([P, T + NL], fp32, tag="labs")
            nc.scalar.dma_start(out=labs[:], in_=labels_t[:])
            pre_fx = {}
            for ci in range(NPRE):
                tc0, L = chunks[ci]
                fx = fxp.tile([P, CH * W], bf16, tag="fx")
                nc.scalar.dma_start(out=fx[:, :L * W],
                                    in_=feats_t[:, tc0 * W:(tc0 + L) * W])
                pre_fx[ci] = fx
            iota32 = labs[:, T:T + NL]

            # raw per-group accumulator dump, finished on host:
            # cols [sumsq_hi, sumsq_lo]*NCH + [norm2]*NCH
            out_t = postp.tile([P, 6], fp32, tag="out_t")

            def post_norm2(c):
                # on DVE (not ACT Square): keeping the scalar engine free of
                # activation ops drops the 1.3us ACT table load from its
                # preamble, so its DMA queue starts earlier. DVE can read at
                # most one PSUM operand, so stage through SBUF first.
                stg = scrp.tile([P, D], fp32, tag="pstg")
                nc.vector.tensor_copy(out=stg[:], in_=ps_sums[c][:])
                scr2 = scrp.tile([P, D], fp32, tag="pscr")
                nc.vector.scalar_tensor_tensor(
                    out=scr2[:], in0=stg[:], scalar=1.0,
                    in1=stg[:], op0=Alu.mult, op1=Alu.mult,
                    accum_out=out_t[:, 4 + c:5 + c])

            def post_small(c):
                nc.vector.tensor_copy(out=out_t[:, 2 * c:2 * c + 2],
                                      in_=ps_small[c][:])

            # DRAM scratch for warming the output-DMA path mid-loop
            warm_dram = dramp.tile([1, 2], fp32)

            def tile_body(ti, X):
                c = 0 if ti < T0 else 1
                # one-hot of this tile's labels vs the active group chunk
                oh16 = oh16p.tile([P, P], bf16, tag="oh16")
                nc.vector.tensor_scalar(
                    out=oh16[:], in0=iota32[:, c * P:(c + 1) * P],
                    scalar1=labs[:, ti:ti + 1], scalar2=None,
                    op0=Alu.is_equal)
                # row sums + group sumsq off the same stationary one-hot
                nc.tensor.matmul(out=ps_sums[c][:], lhsT=oh16[:],
                                 rhs=X[:, 0:D], start=ti in chunk_start,
                                 stop=ti in chunk_stop)
                nc.tensor.matmul(out=ps_small[c][:], lhsT=oh16[:],
                                 rhs=X[:, D:W], start=ti in chunk_start,
                                 stop=ti in chunk_stop)
                if ti in chunk_stop:
                    post_norm2(c)
                    post_small(c)

            warmed = False
            for ci, (t, L) in enumerate(chunks):
                if ci < NPRE:
                    fx = pre_fx[ci]
                else:
                    fx = fxp.tile([P, CH * W], bf16, tag="fx")
                    # alternate mid-loop chunks between the two HW queues
                    q = nc.sync if ci % 2 == 0 else nc.scalar
                    q.dma_start(out=fx[:, :L * W],
                                in_=feats_t[:, t * W:(t + L) * W])
                if not warmed and t + L >= T - 2 * CH:
                    # keep the output-DMA engine hot for the final nd store
                    nc.sync.dma_start(out=warm_dram[:], in_=labs[:1, :2])
                    warmed = True
                for j in range(L):
                    tile_body(t + j, fx[:, j * W:(j + 1) * W])

            nc.sync.dma_start(out=nd_out[:], in_=out_t[:])

    nc.compile()
    return nc


def _shard(feats, labels, demog):
    """Partition rows by (demog, label-half) -> core 2d+h; within each core
    order rows by PSUM chunk (local label < 128 first), padding each chunk
    section to the compile-time tile counts (T0, T1). feats are rounded to
    bf16 here and each row is extended with the bf16 (hi, lo) split of its
    exact-fp32 squared norm."""
    fb = _bf16(feats)
    f32 = fb.astype(np.uint32) << 16
    norms = np.einsum('nd,nd->n', f32.view(np.float32).astype(np.float64),
                      f32.view(np.float32).astype(np.float64))
    norms = norms.astype(np.float32)
    hi = _bf16(norms)
    lo = _bf16(norms - ((hi.astype(np.uint32) << 16).view(np.float32)))
    rows = np.concatenate([fb, hi[:, None], lo[:, None]], axis=1)  # [N, W]

    half = (labels >= NL).astype(np.int32)
    shard_id = demog * 2 + half
    loc = labels % NL
    chunk = loc // P
    parts = []  # per core: (rows_chunk0, rows_chunk1)
    for s in range(N_CORES):
        in_s = shard_id == s
        parts.append((np.flatnonzero(in_s & (chunk == 0)),
                      np.flatnonzero(in_s & (chunk == 1))))
    T0 = max(1, max(-(-len(p[0]) // P) for p in parts))
    T1 = max(1, max(-(-len(p[1]) // P) for p in parts))
    T = T0 + T1
    S = T * P
    in_maps = []
    cnts = []
    for r0, r1 in parts:
        f = np.zeros((S, W), np.uint16)
        lab = np.full(S, 999.0, np.float32)  # pad label matches no group
        f[:len(r0)] = rows[r0]
        lab[:len(r0)] = loc[r0]
        f[T0 * P:T0 * P + len(r1)] = rows[r1]
        lab[T0 * P:T0 * P + len(r1)] = loc[r1]
        # [S, W] -> [P, T*W]: partition p holds its rows contiguously so
        # every DMA descriptor is a fat contiguous run
        ft = np.ascontiguousarray(
            f.reshape(T, P, W).transpose(1, 0, 2).reshape(P, T * W))
        lt = np.ascontiguousarray(np.concatenate(
            [lab.reshape(T, P).T,
             np.tile(np.arange(NL, dtype=np.float32), (P, 1))], axis=1))
        try:
            import ml_dtypes
            ft = ft.view(ml_dtypes.bfloat16)
        except ImportError:
            pass
        in_maps.append({"feats_t": ft, "labels_t": lt})
        # per-(partition-row, chunk) group counts, [P, NCH], same layout
        # as the device accumulators (group g of chunk c <-> partition g)
        cnt = np.zeros((P, NCH), np.float32)
        np.add.at(cnt, (loc[r0] % P, np.zeros(len(r0), np.intp)), 1.0)
        np.add.at(cnt, (loc[r1] % P, np.ones(len(r1), np.intp)), 1.0)
        cnts.append(cnt)
    return (T0, T1), in_maps, cnts


def kernel(feats, labels, demog_labels, _results_out=None):
    feats = np.ascontiguousarray(np.asarray(feats), dtype=np.float32)
    labels = np.asarray(labels).astype(np.int32)
    demog = np.asarray(demog_labels).astype(np.int32)
    assert feats.ndim == 2 and feats.shape[1] == D

    key, in_maps, cnts = _shard(feats, labels, demog)
    nc = _cache.get(key)
    if nc is None:
        nc = _cache.setdefault(key, _build(key))
    res = None
    last_exc = None
    for attempt in range(3):
        try:
            res = bass_utils.run_bass_kernel_spmd(
                nc, in_maps, core_ids=list(range(N_CORES)))
            break
        except Exception as e:  # transient axon worker hangups
            last_exc = e
            import time
            time.sleep(10)
    if res is None:
        raise last_exc
    if _results_out is not None:
        _results_out.append(res)
    num = np.zeros(ND)
    den = np.zeros(ND)
    for i in range(N_CORES):
        nd = np.asarray(res.results[i]["nd"], dtype=np.float32)  # [P, 6]
        sumsq = nd[:, 0::2][:, :NCH] + nd[:, 1::2][:, :NCH]  # hi+lo, [P,NCH]
        norm2 = nd[:, 4:6]
        cnt = cnts[i]
        safe = np.maximum(cnt, 1.0)
        grp = (sumsq - norm2 / safe) / safe
        pres = (cnt > 0)
        num[i // 2] += np.sum(grp[pres])
        den[i // 2] += np.count_nonzero(pres)
    intra = num / np.maximum(den, 1.0)
    loss = np.mean(np.abs(intra - np.mean(intra)))
    return np.float32(loss)
